# revision 3
# baseline (speedup 1.0000x reference)
"""GCN2 (gnn_message_passing) Trainium2 kernel for nn_Net_76665166234219.

Strategy: nodes sharded 8 ways by dst; h (both branches fused, 128 feats)
replicated in each core's DRAM; per-edge rows fetched with dma_gather
(4 src blocks to satisfy int16 indices); segment reduction via one-hot
matmul on TensorE accumulating feature-major agg tiles in PSUM; GCN2
update folded into two 128x128 matmuls per node chunk; per-layer
AllGather re-replicates h.
"""
import math
import sys
import types

import numpy as np

NCORES = 8
N = 100000
NPC = 12500          # real nodes per core
NPCP = 12544         # padded per-core nodes (98 * 128)
NTILES = NPCP // 128  # 98
NBLK = 4
BLK = 2 * NPCP       # src block rows (25088), int16-safe
H = 128              # fused feature width (64 + 64)
L = 4
ALPHA = 0.4
THETA = 0.9
F_IN = 512
F_STR = 58
CHUNK_RANKS = 16     # ranks (128 edges each) per dma_gather chunk

_LAST = {}


def _patch_env():
    """Walrus drain-wait workaround + NTFF profile hook shim."""
    import concourse.tile as tile
    from concourse.vector_clock import ScopedClock
    import bass_rust

    def _drain_and_barrier(self, tick_clock, wait_clock):
        nc = self.nc
        drain_inst = nc.sync.drain()
        wait_clock.add_sem_waits(
            drain_inst.ins, ScopedClock({None: tick_clock.global_clock})
        )
        si = drain_inst.ins.sync_info
        waits = list(si.on_wait) if si is not None else []
        if len(waits) > 1:
            si.on_wait = waits[:1]
            for i in range(1, len(waits)):
                extra = nc.sync.drain()
                if extra.ins.sync_info is None:
                    extra.ins.sync_info = bass_rust.SyncInfo(
                        on_wait=waits[i : i + 1], on_update=[]
                    )
                else:
                    extra.ins.sync_info.on_wait = waits[i : i + 1]
        nc.all_engine_barrier()
        assert self.sems is not None
        popped = nc._tile_sem_poison_stack.pop()
        assert popped is self._sem_poison
        nc.clear_and_free_semaphores(list(self.sems.allocated().values()))
        nc.all_engine_barrier()

    tile.TileContext._drain_and_barrier = _drain_and_barrier
    try:
        import antenv.axon_hooks  # noqa: F401
    except ImportError:
        try:
            from trn_agent_boot.trn_boot import _ntff_profile_via_ctypes

            mod = types.ModuleType("antenv.axon_hooks")
            hook = _ntff_profile_via_ctypes("/opt/axon/libaxon_pjrt.so")
            mod.get_axon_ntff_profile_hook = lambda: hook
            mod.set_axon_ntff_profile_hook = lambda h: None
            sys.modules["antenv.axon_hooks"] = mod
        except Exception:
            pass


def _row_of(n):
    return (n // NPC) * NPCP + (n % NPC)


def _prep(edge_index):
    """Host edge preprocessing -> per-core arrays + shared schedule."""
    src = np.asarray(edge_index[0], dtype=np.int64)
    dst = np.asarray(edge_index[1], dtype=np.int64)
    deg = np.bincount(dst, minlength=N).astype(np.float64)
    dinv = np.where(deg > 0, 1.0 / np.sqrt(np.maximum(deg, 1.0)), 0.0)
    norm = (dinv[src] * dinv[dst]).astype(np.float32)

    srcp = _row_of(src)
    blk = (srcp // BLK).astype(np.int64)
    src_local = (srcp % BLK).astype(np.int16)
    core = (dst // NPC).astype(np.int64)
    dst_local = (dst % NPC).astype(np.int64)
    tilei = (dst_local // 128).astype(np.int64)
    dstl = (dst_local % 128).astype(np.float32)

    key_all = (core * NBLK + blk) * NTILES + tilei
    counts = np.bincount(key_all, minlength=NCORES * NBLK * NTILES).reshape(
        NCORES, NBLK, NTILES
    )
    r_bt = np.ceil(counts.max(axis=0) / 128).astype(np.int64)  # [NBLK, NTILES]
    ranks_b = r_bt.sum(axis=1)
    pad_b = (-ranks_b) % CHUNK_RANKS
    r_bt[:, NTILES - 1] += pad_b  # pad ranks land in last tile with norm 0
    ranks_b = r_bt.sum(axis=1)
    rtot = int(ranks_b.sum())
    eslots = rtot * 128

    off_bt = np.zeros((NBLK, NTILES), np.int64)
    acc = 0
    for b in range(NBLK):
        for t in range(NTILES):
            off_bt[b, t] = acc
            acc += r_bt[b, t] * 128

    idx_all = np.zeros((NCORES, eslots), np.int16)
    dstl_all = np.zeros((NCORES, eslots), np.float32)
    norm_all = np.zeros((NCORES, eslots), np.float32)

    order = np.lexsort((tilei, blk, core))
    so_src, so_dstl, so_norm = src_local[order], dstl[order], norm[order]
    so_key = key_all[order]
    starts = np.searchsorted(so_key, np.arange(NCORES * NBLK * NTILES))
    ends = np.searchsorted(so_key, np.arange(NCORES * NBLK * NTILES) + 1)
    for c in range(NCORES):
        for b in range(NBLK):
            base = (c * NBLK + b) * NTILES
            for t in range(NTILES):
                s, e = starts[base + t], ends[base + t]
                n_ = e - s
                if n_ == 0:
                    continue
                o = off_bt[b, t]
                idx_all[c, o : o + n_] = so_src[s:e]
                dstl_all[c, o : o + n_] = so_dstl[s:e]
                norm_all[c, o : o + n_] = so_norm[s:e]

    idx_w = np.ascontiguousarray(
        idx_all.reshape(NCORES, eslots // 16, 16).transpose(0, 2, 1)
    )  # [C,16,eslots/16]
    idx_w = np.tile(idx_w, (1, 8, 1))  # [C,128,eslots/16]
    dstl_w = np.ascontiguousarray(
        dstl_all.reshape(NCORES, rtot, 128).transpose(0, 2, 1)
    )  # [C,128,rtot]
    norm_w = np.ascontiguousarray(
        norm_all.reshape(NCORES, rtot, 128).transpose(0, 2, 1)
    )

    sched = []  # per block: list of (tile, nranks)
    for b in range(NBLK):
        sched.append([(t, int(r_bt[b, t])) for t in range(NTILES) if r_bt[b, t] > 0])
    return dict(
        idx_w=idx_w, dstl_w=dstl_w, norm_w=norm_w, sched=sched,
        ranks_b=[int(x) for x in ranks_b], rtot=rtot, eslots=eslots,
    )


def _weights(inp):
    """Fold alpha/beta/identity into per-layer fused 128x128 weights."""
    f32 = np.float32
    w1f = np.zeros((L, H, H), f32)
    w2f = np.zeros((L, H, H), f32)
    eye = np.eye(H, dtype=f32)
    for l in range(L):
        beta = math.log(THETA / (l + 1) + 1.0)
        w1 = np.zeros((H, H), f32)
        w1[:64, :64] = inp["conv_w1"][l]
        w1[64:, 64:] = inp["conv1_w1"][l]
        w2 = np.zeros((H, H), f32)
        w2[:64, :64] = inp["conv_w2"][l]
        w2[64:, 64:] = inp["conv1_w2"][l]
        w1f[l] = (1.0 - ALPHA) * ((1.0 - beta) * eye + beta * w1)
        w2f[l] = ALPHA * ((1.0 - beta) * eye + beta * w2)
    w0t = np.ascontiguousarray(
        np.asarray(inp["W_lin0"], f32).T.reshape(4, 128, 64)
    )
    w11t = np.ascontiguousarray(np.asarray(inp["W_lin11"], f32).T)  # [58,64]
    wz = np.zeros((H, 2), f32)
    wz[:64, 0] = np.asarray(inp["W_lins1"], f32)[0]
    wz[64:, 1] = np.asarray(inp["W_lin3"], f32)[0]
    biascol = np.zeros((H, 1), f32)
    biascol[:64, 0] = np.asarray(inp["b_lin0"], f32)
    biascol[64:, 0] = np.asarray(inp["b_lin11"], f32)
    return w1f, w2f, w0t, w11t, wz, biascol


def _build(meta):
    import concourse.bacc as bacc
    import concourse.mybir as mybir
    import concourse.tile as tile

    f32 = mybir.dt.float32
    nc = bacc.Bacc(None, num_devices=NCORES, num_swdge_queues=4)

    xt = nc.dram_tensor("xt", [F_IN, NPCP], f32, kind="ExternalInput")
    xst = nc.dram_tensor("xst", [F_STR, NPCP], f32, kind="ExternalInput")
    idx_d = nc.dram_tensor("idx", [128, meta["eslots"] // 16], mybir.dt.int16, kind="ExternalInput")
    dstl_d = nc.dram_tensor("dstl", [128, meta["rtot"]], f32, kind="ExternalInput")
    norm_d = nc.dram_tensor("norm", [128, meta["rtot"]], f32, kind="ExternalInput")
    w1f_d = nc.dram_tensor("w1f", [L, H, H], f32, kind="ExternalInput")
    w2f_d = nc.dram_tensor("w2f", [L, H, H], f32, kind="ExternalInput")
    w0t_d = nc.dram_tensor("w0t", [4, 128, 64], f32, kind="ExternalInput")
    w11t_d = nc.dram_tensor("w11t", [F_STR, 64], f32, kind="ExternalInput")
    wz_d = nc.dram_tensor("wz", [H, 2], f32, kind="ExternalInput")
    bias_d = nc.dram_tensor("biascol", [H, 1], f32, kind="ExternalInput")
    iota_d = nc.dram_tensor("iota", [128, 128], f32, kind="ExternalInput")
    eye_d = nc.dram_tensor("eye", [128, 128], f32, kind="ExternalInput")
    out_z = nc.dram_tensor("out_z", [2, NPCP], f32, kind="ExternalOutput")

    sched = meta["sched"]
    ranks_b = meta["ranks_b"]

    node_chunks = []
    c0 = 0
    while c0 < NPCP:
        w = min(512, NPCP - c0)
        node_chunks.append((c0, w))
        c0 += w

    CR = CHUNK_RANKS

    with tile.TileContext(nc) as tc:
        with tc.tile_pool(name="const", bufs=1) as constp, \
             tc.tile_pool(name="resid", bufs=1) as resid, \
             tc.tile_pool(name="edge", bufs=1) as edgep, \
             tc.tile_pool(name="idxp", bufs=4) as idxp, \
             tc.tile_pool(name="vload", bufs=2) as vpool, \
             tc.tile_pool(name="sbuild", bufs=2) as spool, \
             tc.tile_pool(name="xload", bufs=2) as xpool, \
             tc.tile_pool(name="upd", bufs=2) as updp, \
             tc.tile_pool(name="stage", bufs=4) as stagep, \
             tc.tile_pool(name="ps_agg", bufs=3, space="PSUM") as ps_aggp, \
             tc.tile_pool(name="ps_tr", bufs=2, space="PSUM") as ps_trp, \
             tc.tile_pool(name="ps_z", bufs=1, space="PSUM") as ps_zp, \
             tc.tile_pool(name="ps_u", bufs=2, space="PSUM") as ps_up, \
             tc.tile_pool(name="dram", bufs=1, space="DRAM") as dram:

            # ---- constants ----
            iota_t = constp.tile([128, 128], f32)
            nc.sync.dma_start(out=iota_t[:], in_=iota_d[:])
            eye_t = constp.tile([128, 128], f32)
            nc.sync.dma_start(out=eye_t[:], in_=eye_d[:])
            w1f_t = constp.tile([128, L, H], f32)
            nc.sync.dma_start(out=w1f_t[:], in_=w1f_d[:].rearrange('l p h -> p l h'))
            w2f_t = constp.tile([128, L, H], f32)
            nc.sync.dma_start(out=w2f_t[:], in_=w2f_d[:].rearrange('l p h -> p l h'))
            w0t_t = constp.tile([128, 4, 64], f32)
            nc.sync.dma_start(out=w0t_t[:], in_=w0t_d[:].rearrange('k p h -> p k h'))
            w11t_t = constp.tile([F_STR, 64], f32)
            nc.sync.dma_start(out=w11t_t[:], in_=w11t_d[:])
            wz_t = constp.tile([128, 2], f32)
            nc.sync.dma_start(out=wz_t[:], in_=wz_d[:])
            bias_t = constp.tile([128, 1], f32)
            nc.sync.dma_start(out=bias_t[:], in_=bias_d[:])

            dstl_sb = edgep.tile([128, meta["rtot"]], f32)
            nc.sync.dma_start(out=dstl_sb[:], in_=dstl_d[:])
            norm_sb = edgep.tile([128, meta["rtot"]], f32)
            nc.sync.dma_start(out=norm_sb[:], in_=norm_d[:])

            h0T = resid.tile([128, NPCP], f32)
            aggT = resid.tile([128, NPCP], f32)
            z_sb = resid.tile([2, NPCP], f32)

            h_full = [dram.tile([NBLK * BLK, H], f32, tag=f"hf{i}", name=f"hfull{i}") for i in range(2)]
            slice_d = dram.tile([NPCP, H], f32)

            def emit_transpose_out(srcT, col0, ncols):
                for tt in range(ncols // 128):
                    pt = ps_trp.tile([128, 128], f32, tag="ps_tr")
                    nc.tensor.transpose(
                        out=pt[:],
                        in_=srcT[:, col0 + tt * 128:col0 + (tt + 1) * 128],
                        identity=eye_t[:],
                    )
                    st = stagep.tile([128, 128], f32, tag="tr_stage")
                    nc.vector.tensor_copy(out=st[:], in_=pt[:])
                    nc.sync.dma_start(
                        out=slice_d[col0 + tt * 128:col0 + (tt + 1) * 128, :],
                        in_=st[:],
                    )

            # ---- prologue: h0 = relu([x@W0.T | x_str@W11.T] + b) ----
            for (cc, w) in node_chunks:
                ps = ps_up.tile([128, 512], f32, tag="ps_u")
                for k in range(4):
                    xtile = xpool.tile([128, 512], f32, tag="xt")
                    nc.sync.dma_start(
                        out=xtile[:, :w], in_=xt[k * 128:(k + 1) * 128, cc:cc + w]
                    )
                    nc.tensor.matmul(
                        out=ps[0:64, :w], lhsT=w0t_t[:, k, :], rhs=xtile[:, :w],
                        start=(k == 0), stop=(k == 3),
                    )
                xstile = xpool.tile([F_STR, 512], f32, tag="xst")
                nc.sync.dma_start(out=xstile[:, :w], in_=xst[:, cc:cc + w])
                nc.tensor.matmul(
                    out=ps[64:128, :w], lhsT=w11t_t[:], rhs=xstile[:, :w],
                    start=True, stop=True,
                )
                nc.vector.tensor_scalar(
                    out=h0T[:, cc:cc + w], in0=ps[:, :w],
                    scalar1=bias_t[:], scalar2=0.0,
                    op0=mybir.AluOpType.add, op1=mybir.AluOpType.max,
                )
                emit_transpose_out(h0T, cc, (w // 128) * 128)
            nc.gpsimd.collective_compute(
                "AllGather", mybir.AluOpType.bypass,
                replica_groups=[list(range(NCORES))],
                ins=[slice_d.opt()], outs=[h_full[0].opt()],
            )

            # ---- layers ----
            for l in range(L):
                hsrc = h_full[l % 2]
                nc.vector.memset(aggT[:], 0.0)
                rank_base = 0
                for b in range(NBLK):
                    # flat walk: chunks interleaved with their groups' matmuls
                    groups = sched[b]
                    gi = 0            # current group index
                    k_in_g = 0        # ranks of current group already consumed
                    ps = None
                    nch = ranks_b[b] // CR
                    for ch in range(nch):
                        r0 = rank_base + ch * CR
                        idxt = idxp.tile([128, CR * 8], mybir.dt.int16, tag="idxt")
                        nc.sync.dma_start(
                            out=idxt[:], in_=idx_d[:, r0 * 8:(r0 + CR) * 8]
                        )
                        V = vpool.tile([128, CR, 128], f32, tag="V")
                        nc.gpsimd.dma_gather(
                            V[:], hsrc[b * BLK:(b + 1) * BLK, :],
                            idxt[:], CR * 128, CR * 128, H,
                            single_packet=False, queue_num=ch % 4,
                        )
                        nc.vector.tensor_tensor(
                            out=V[:], in0=V[:],
                            in1=norm_sb[:, r0:r0 + CR].unsqueeze(2).to_broadcast(
                                [128, CR, 128]
                            ),
                            op=mybir.AluOpType.mult,
                        )
                        S = spool.tile([128, CR, 128], f32, tag="S")
                        nc.vector.tensor_tensor(
                            out=S[:],
                            in0=iota_t[:].unsqueeze(1).to_broadcast([128, CR, 128]),
                            in1=dstl_sb[:, r0:r0 + CR].unsqueeze(2).to_broadcast(
                                [128, CR, 128]
                            ),
                            op=mybir.AluOpType.is_equal,
                        )
                        for rr in range(CR):
                            t, nr = groups[gi]
                            if k_in_g == 0:
                                ps = ps_aggp.tile([128, 128], f32, tag="ps_agg")
                            nc.tensor.matmul(
                                out=ps[:], lhsT=V[:, rr, :], rhs=S[:, rr, :],
                                start=(k_in_g == 0), stop=(k_in_g == nr - 1),
                            )
                            k_in_g += 1
                            if k_in_g == nr:
                                nc.vector.tensor_tensor(
                                    out=aggT[:, t * 128:(t + 1) * 128],
                                    in0=aggT[:, t * 128:(t + 1) * 128],
                                    in1=ps[:], op=mybir.AluOpType.add,
                                )
                                gi += 1
                                k_in_g = 0
                    assert gi == len(groups) and k_in_g == 0, (b, gi, k_in_g)
                    rank_base += ranks_b[b]

                # update
                for (cc, w) in node_chunks:
                    ps = ps_up.tile([128, 512], f32, tag="ps_u")
                    nc.tensor.matmul(
                        out=ps[:, :w], lhsT=w1f_t[:, l, :], rhs=aggT[:, cc:cc + w],
                        start=True, stop=False,
                    )
                    nc.tensor.matmul(
                        out=ps[:, :w], lhsT=w2f_t[:, l, :], rhs=h0T[:, cc:cc + w],
                        start=False, stop=True,
                    )
                    hT = updp.tile([128, 512], f32, tag="hT")
                    nc.vector.tensor_scalar_max(out=hT[:, :w], in0=ps[:, :w], scalar1=0.0)
                    if l < L - 1:
                        for tt in range(w // 128):
                            pt = ps_trp.tile([128, 128], f32, tag="ps_tr")
                            nc.tensor.transpose(
                                out=pt[:], in_=hT[:, tt * 128:(tt + 1) * 128],
                                identity=eye_t[:],
                            )
                            st = stagep.tile([128, 128], f32, tag="tr_stage")
                            nc.vector.tensor_copy(out=st[:], in_=pt[:])
                            nc.sync.dma_start(
                                out=slice_d[cc + tt * 128:cc + (tt + 1) * 128, :],
                                in_=st[:],
                            )
                    else:
                        pz = ps_zp.tile([2, 512], f32, tag="ps_z")
                        nc.tensor.matmul(
                            out=pz[:, :w], lhsT=wz_t[:], rhs=hT[:, :w],
                            start=True, stop=True,
                        )
                        nc.vector.tensor_copy(out=z_sb[:, cc:cc + w], in_=pz[:, :w])
                if l < L - 1:
                    nc.gpsimd.collective_compute(
                        "AllGather", mybir.AluOpType.bypass,
                        replica_groups=[list(range(NCORES))],
                        ins=[slice_d.opt()], outs=[h_full[(l + 1) % 2].opt()],
                    )

            nc.sync.dma_start(out=out_z[:], in_=z_sb[:])

    nc.compile()
    return nc


def kernel(**inputs):
    _patch_env()
    from concourse.bass_utils import run_bass_kernel_spmd

    meta = _prep(np.asarray(inputs["edge_index"]))
    w1f, w2f, w0t, w11t, wz, biascol = _weights(
        {k: np.asarray(v) for k, v in inputs.items()}
    )
    x = np.asarray(inputs["x"], np.float32)
    x_str = np.asarray(inputs["x_str"], np.float32)

    iota = np.tile(np.arange(128, dtype=np.float32)[None, :], (128, 1))
    eye = np.eye(128, dtype=np.float32)

    in_maps = []
    for c in range(NCORES):
        xs = np.zeros((F_IN, NPCP), np.float32)
        xs[:, :NPC] = x[c * NPC:(c + 1) * NPC].T
        xss = np.zeros((F_STR, NPCP), np.float32)
        xss[:, :NPC] = x_str[c * NPC:(c + 1) * NPC].T
        in_maps.append({
            "xt": xs, "xst": xss,
            "idx": meta["idx_w"][c], "dstl": meta["dstl_w"][c],
            "norm": meta["norm_w"][c],
            "w1f": w1f, "w2f": w2f, "w0t": w0t, "w11t": w11t, "wz": wz,
            "biascol": biascol, "iota": iota, "eye": eye,
        })

    nc = _build(meta)
    trace = bool(_LAST.get("want_trace", True))
    res = run_bass_kernel_spmd(nc, in_maps, list(range(NCORES)), trace=trace)
    _LAST["exec_time_ns"] = res.exec_time_ns
    _LAST["res"] = res

    z = np.zeros((N, 1), np.float32)
    z1 = np.zeros((N, 1), np.float32)
    for c in range(NCORES):
        oz = res.results[c]["out_z"]
        z[c * NPC:(c + 1) * NPC, 0] = oz[0, :NPC]
        z1[c * NPC:(c + 1) * NPC, 0] = oz[1, :NPC]
    z += np.asarray(inputs["b_lins1"], np.float32)[None, :]
    z1 += np.asarray(inputs["b_lin3"], np.float32)[None, :]
    return z, z1
